# revision 8
# baseline (speedup 1.0000x reference)
"""Trainium2 Bass kernel for nn_Attention_34119220199768 (sparse attention).

Data-parallel over batch B=16 across 8 NeuronCores (2 batches/core).

Per batch b the reference computes
    qs  = query @ Wq + bq            [S,W,Din]
    k_  = data @ Wk + bk             [S,Din]
    vs  = data @ Wv + bv             [S,Dout]
    att = softmax_k(mask(qs . k_ / sqrt(Din)))   [q,k,w]
    zs  = (att @ vs) @ Wz + bz       [S,W,Dout]

Device-side restructuring (everything fp32; matmuls in fp32r = full-rate):
    kT   = Wk^T @ dataT  (+bk)       [Din,S]    (dataT host-pretransposed)
    vsT  = Wv^T @ dataT  (+bv)       [Dout,S]
    WqK  = Wq @ kT                   [DQ,S]     -- scores = query@(Wq@kT)
    bqk  = bq @ kT                   [1,S]
    VZ   = vsT^T-contract Wz         [S,Dout]   -- zs = P@(vs@Wz)
    per 128-row tile of (q,w):
      s   = queryT^T @ WqK (+bqk via K=1 matmul)         [128,S]
      t   = (s + KMASK)*cm*wm  (masked, shifted)         [128,S]
      P   = exp((t - max_k t)/sqrt(Din))  (unnormalized) [128,S]
      PT  = P^T  (TensorE transpose)                     [S,128]
      zs_un = PT^T-contract VZ                            [128,Dout]
Host finishes: row sums, att = P/sum, zs = zs_un/sum + bz.
Fully-masked rows come out uniform (P=1 each, sum=S) exactly like the
reference's softmax over all -1e9.
"""

import numpy as np

import concourse.bass as bass
import concourse.mybir as mybir
import concourse.tile as tile
from concourse import bacc
from concourse import bass_utils
from concourse.masks import make_identity

F32 = mybir.dt.float32
F32R = mybir.dt.float32r

N_CORES = 8
B, S, W, DQ, DIN, DOUT = 16, 512, 8, 256, 512, 512
BPC = B // N_CORES          # batches per core
QW = S * W                  # 4096 flattened (q, w) rows
K_MASK = 3400.0             # shift: exp((0 - (K+max_s))/sqrt(DIN)) underflows to 0
BIG_WM = 1.0e5              # column penalty folded into bqk for wm-masked keys
INV = 1.0 / float(np.sqrt(np.float32(DIN)))

N_SUB = QW // 128           # 32 row-subtiles per batch
SUB_PER_CH = 4              # subtiles per query chunk
N_CH = N_SUB // SUB_PER_CH  # 8 chunks per batch


def build():
    nc = bacc.Bacc("TRN2", target_bir_lowering=False, debug=False,
                   num_devices=N_CORES)

    def din(name, shape, dt=F32R):
        return nc.dram_tensor(name, shape, dt, kind="ExternalInput").ap()

    qT = din("queryT", [BPC, DQ, QW])
    dT = din("dataT", [BPC, DIN, S])
    wqt = din("WqT", [DIN, DQ])
    wk = din("Wk", [DIN, DIN])
    wv = din("Wv", [DIN, DOUT])
    wz = din("Wz", [DOUT, DOUT])
    bq = din("bq", [DIN])
    bk = din("bk", [DIN], F32)
    bv = din("bv", [DOUT], F32)
    cm = din("cm", [BPC, QW], F32)
    wm = din("wm", [BPC, S], F32)
    att_o = nc.dram_tensor("att", [BPC, QW, S], F32, kind="ExternalOutput").ap()
    zs_o = nc.dram_tensor("zs", [BPC, QW, DOUT], F32, kind="ExternalOutput").ap()

    from contextlib import ExitStack
    with tile.TileContext(nc) as tc, ExitStack() as ctx:
        sing = ctx.enter_context(tc.tile_pool(name="sing", bufs=1))
        batch_p = ctx.enter_context(tc.tile_pool(name="batch", bufs=2))
        chunk_p = ctx.enter_context(tc.tile_pool(name="chunk", bufs=3))
        sub_p = ctx.enter_context(tc.tile_pool(name="sub", bufs=3))
        stat_p = ctx.enter_context(tc.tile_pool(name="stat", bufs=6))
        ps_set = ctx.enter_context(tc.tile_pool(name="ps_set", bufs=2, space="PSUM"))
        ps_sc = ctx.enter_context(tc.tile_pool(name="ps_sc", bufs=2, space="PSUM"))
        ps_pt = ctx.enter_context(tc.tile_pool(name="ps_pt", bufs=2, space="PSUM"))
        ps_zs = ctx.enter_context(tc.tile_pool(name="ps_zs", bufs=2, space="PSUM"))

        # ---- one-time weights / constants ----
        wqt_s = sing.tile([128, 4, DQ], F32R)
        nc.sync.dma_start(out=wqt_s, in_=wqt.rearrange("(t p) d -> p t d", p=128))
        wk_s = sing.tile([128, 4, DIN], F32R)
        nc.sync.dma_start(out=wk_s, in_=wk.rearrange("(t p) i -> p t i", p=128))
        wv_s = sing.tile([128, 4, DOUT], F32R)
        nc.sync.dma_start(out=wv_s, in_=wv.rearrange("(t p) z -> p t z", p=128))
        wz_s = sing.tile([128, 4, DOUT], F32R)
        nc.sync.dma_start(out=wz_s, in_=wz.rearrange("(t p) z -> p t z", p=128))
        bq_s = sing.tile([128, 4], F32R)
        nc.sync.dma_start(out=bq_s, in_=bq.rearrange("(t p) -> p t", p=128))
        bk_s = sing.tile([128, 4], F32)
        nc.sync.dma_start(out=bk_s, in_=bk.rearrange("(t p) -> p t", p=128))
        bv_s = sing.tile([128, 4], F32)
        nc.sync.dma_start(out=bv_s, in_=bv.rearrange("(t p) -> p t", p=128))
        ident = sing.tile([128, 128], F32)
        make_identity(nc, ident)
        ones_f = sing.tile([1, 128], F32)
        nc.vector.memset(ones_f, 1.0)
        ones_r = sing.tile([1, 128], F32R)
        nc.vector.tensor_copy(ones_r, ones_f)

        for b in range(BPC):
            # ---- batch setup ----
            dT_s = batch_p.tile([128, 4, S], F32R)
            nc.sync.dma_start(out=dT_s,
                              in_=dT[b].rearrange("(t p) k -> p t k", p=128))
            wm_row = batch_p.tile([1, S], F32)
            nc.sync.dma_start(out=wm_row, in_=wm[b][None, :])
            wmpen = batch_p.tile([1, S], F32)
            nc.vector.tensor_scalar(wmpen, wm_row, BIG_WM, -BIG_WM,
                                    op0=mybir.AluOpType.mult,
                                    op1=mybir.AluOpType.add)
            cm_s = batch_p.tile([128, N_SUB], F32)
            nc.sync.dma_start(out=cm_s,
                              in_=cm[b].rearrange("(t p) -> p t", p=128))
            cmI_s = batch_p.tile([128, N_SUB], F32)
            nc.vector.tensor_scalar_mul(cmI_s, cm_s, INV)

            # kT[i,k] = sum_j Wk[j,i] dataT[j,k]  (+bk per-partition)
            kT_s = batch_p.tile([128, 4, S], F32R)
            for it in range(4):
                ps = ps_set.tile([128, S], F32)
                for jt in range(4):
                    nc.tensor.matmul(ps, wk_s[:, jt, it * 128:(it + 1) * 128],
                                     dT_s[:, jt, :],
                                     start=(jt == 0), stop=(jt == 3))
                nc.vector.tensor_scalar_add(kT_s[:, it, :], ps,
                                            bk_s[:, it:it + 1])
            # vsT[z,k] = sum_j Wv[j,z] dataT[j,k]  (+bv per-partition)
            vsT_s = batch_p.tile([128, 4, S], F32R)
            for zt in range(4):
                ps = ps_set.tile([128, S], F32)
                for jt in range(4):
                    nc.tensor.matmul(ps, wv_s[:, jt, zt * 128:(zt + 1) * 128],
                                     dT_s[:, jt, :],
                                     start=(jt == 0), stop=(jt == 3))
                nc.vector.tensor_scalar_add(vsT_s[:, zt, :], ps,
                                            bv_s[:, zt:zt + 1])
            # WqK[d,k] = sum_i Wq[d,i] kT[i,k]
            wqk_s = batch_p.tile([128, 2, S], F32R)
            for dt_ in range(2):
                ps = ps_set.tile([128, S], F32)
                for it in range(4):
                    nc.tensor.matmul(ps, wqt_s[:, it, dt_ * 128:(dt_ + 1) * 128],
                                     kT_s[:, it, :],
                                     start=(it == 0), stop=(it == 3))
                nc.vector.tensor_copy(wqk_s[:, dt_, :], ps)
            # bqk[1,k] = sum_i bq[i] kT[i,k]  - BIG_WM*(1-wm[k])  (wm fold)
            bqk_s = batch_p.tile([1, S], F32R)
            ps = ps_set.tile([128, S], F32)
            for it in range(4):
                nc.tensor.matmul(ps[0:1, :], bq_s[:, it:it + 1], kT_s[:, it, :],
                                 start=(it == 0), stop=(it == 3))
            nc.vector.tensor_add(bqk_s, ps[0:1, :], wmpen)
            # VZ[k,z2] = sum_z vsT[z,k] Wz[z,z2]
            vz_s = batch_p.tile([128, 4, DOUT], F32R)
            for kt in range(4):
                ps = ps_set.tile([128, DOUT], F32)
                for zt in range(4):
                    nc.tensor.matmul(ps, vsT_s[:, zt, kt * 128:(kt + 1) * 128],
                                     wz_s[:, zt, :],
                                     start=(zt == 0), stop=(zt == 3))
                nc.vector.tensor_copy(vz_s[:, kt, :], ps)

            # ---- main loop over (q,w) row tiles ----
            for ch in range(N_CH):
                qT_c = chunk_p.tile([128, 2, 128 * SUB_PER_CH], F32R)
                nc.sync.dma_start(
                    out=qT_c,
                    in_=qT[b].rearrange("(t p) c -> p t c", p=128)
                    [:, :, ch * 512:(ch + 1) * 512])
                for sl in range(SUB_PER_CH):
                    st = ch * SUB_PER_CH + sl
                    rows = slice(st * 128, (st + 1) * 128)
                    # scores = bqk (K=1) + sum_d queryT^T WqK
                    psc = ps_sc.tile([128, S], F32)
                    nc.tensor.matmul(psc, ones_r, bqk_s, start=True, stop=False)
                    for dt_ in range(2):
                        nc.tensor.matmul(
                            psc, qT_c[:, dt_, sl * 128:(sl + 1) * 128],
                            wqk_s[:, dt_, :], start=False, stop=(dt_ == 1))
                    # P = exp(cm*(s - max_k s)/sqrt(DIN))  straight from PSUM;
                    # wm-masked cols carry -BIG_WM in s (via bqk) -> exp -> 0;
                    # cm=0 rows get scale=bias=0 -> exp(0)=1 (uniform row).
                    mx = stat_p.tile([128, 1], F32)
                    nc.vector.reduce_max(mx, psc, axis=mybir.AxisListType.X,
                                         negate=True)
                    be = stat_p.tile([128, 1], F32)
                    nc.gpsimd.tensor_scalar_mul(be, mx, cmI_s[:, st:st + 1])
                    p_s = sub_p.tile([128, S], F32)
                    nc.scalar.activation(
                        out=p_s, in_=psc,
                        func=mybir.ActivationFunctionType.Exp,
                        bias=be[:, 0:1], scale=cmI_s[:, st:st + 1])
                    nc.sync.dma_start(out=att_o[b, rows, :], in_=p_s)
                    # PT = P^T via TensorE transpose (4x 128x128)
                    ppt = ps_pt.tile([128, S], F32)
                    for kt in range(4):
                        nc.tensor.transpose(
                            ppt[:, kt * 128:(kt + 1) * 128],
                            p_s[:, kt * 128:(kt + 1) * 128], ident)
                    pt_s = sub_p.tile([128, S], F32R)
                    nc.vector.tensor_copy(pt_s, ppt)
                    # zs_un = sum_k PT[k,qw]^T VZ[k,z2]
                    pzs = ps_zs.tile([128, DOUT], F32)
                    for kt in range(4):
                        nc.tensor.matmul(pzs, pt_s[:, kt * 128:(kt + 1) * 128],
                                         vz_s[:, kt, :],
                                         start=(kt == 0), stop=(kt == 3))
                    zs_s = sub_p.tile([128, DOUT], F32)
                    nc.scalar.copy(zs_s, pzs)
                    nc.sync.dma_start(out=zs_o[b, rows, :], in_=zs_s)

    nc.compile()
    return nc


_NC_CACHE = None


def _get_nc():
    global _NC_CACHE
    if _NC_CACHE is None:
        _NC_CACHE = build()
    return _NC_CACHE


def make_in_maps(query, data, content_mask, Wq, bq, Wk, bk, Wv, bv, Wz, bz):
    query = np.ascontiguousarray(query, dtype=np.float32)
    data = np.ascontiguousarray(data, dtype=np.float32)
    cm_full = np.asarray(content_mask).astype(np.float32).reshape(B, QW)
    wm_full = np.asarray(content_mask).any(axis=2).astype(np.float32)
    # device wm-fold assumes every batch has >=1 word-valid key (else the
    # reference row would be uniform while the device row would softmax)
    assert wm_full.any(axis=1).all(), "batch with all-masked words_mask"
    wqt = np.ascontiguousarray(np.asarray(Wq, dtype=np.float32).T)
    in_maps = []
    for c in range(N_CORES):
        sl = slice(c * BPC, (c + 1) * BPC)
        q_c = query[sl].reshape(BPC, QW, DQ)
        in_maps.append({
            "queryT": np.ascontiguousarray(q_c.transpose(0, 2, 1)),
            "dataT": np.ascontiguousarray(data[sl].transpose(0, 2, 1)),
            "WqT": wqt,
            "Wk": np.ascontiguousarray(Wk, dtype=np.float32),
            "Wv": np.ascontiguousarray(Wv, dtype=np.float32),
            "Wz": np.ascontiguousarray(Wz, dtype=np.float32),
            "bq": np.ascontiguousarray(bq, dtype=np.float32),
            "bk": np.ascontiguousarray(bk, dtype=np.float32),
            "bv": np.ascontiguousarray(bv, dtype=np.float32),
            "cm": np.ascontiguousarray(cm_full[sl]),
            "wm": np.ascontiguousarray(wm_full[sl]),
        })
    return in_maps


def postprocess(results, bz):
    """Gather per-core raw outputs -> full (zs, att) with host normalization."""
    bz = np.asarray(bz, dtype=np.float32)
    att_raw = np.concatenate([r["att"] for r in results], axis=0)  # [B,QW,S]
    zs_raw = np.concatenate([r["zs"] for r in results], axis=0)    # [B,QW,Dout]
    sums = att_raw.sum(axis=-1, dtype=np.float32)                  # [B,QW]
    att_n = att_raw / sums[..., None]
    zs_n = zs_raw / sums[..., None] + bz
    # layouts: att [B,q,k,w] from [B,(q,w),k]; zs [B,S,W,Dout]
    att = np.ascontiguousarray(
        att_n.reshape(B, S, W, S).transpose(0, 1, 3, 2)).astype(np.float32)
    zs = zs_n.reshape(B, S, W, DOUT).astype(np.float32)
    return zs, att


def kernel(query, data, content_mask, Wq, bq, Wk, bk, Wv, bv, Wz, bz):
    nc = _get_nc()
    in_maps = make_in_maps(query, data, content_mask, Wq, bq, Wk, bk,
                           Wv, bv, Wz, bz)
    res = bass_utils.run_bass_kernel_spmd(nc, in_maps,
                                          core_ids=list(range(N_CORES)),
                                          trace=False)
    return postprocess(res.results, bz)


# revision 11
# speedup vs baseline: 1.1475x; 1.1475x over previous
"""Trainium2 Bass kernel for nn_Attention_34119220199768 (sparse attention).

Data-parallel over batch B=16 across 8 NeuronCores (2 batches/core).

Per batch b the reference computes
    qs  = query @ Wq + bq            [S,W,Din]
    k_  = data @ Wk + bk             [S,Din]
    vs  = data @ Wv + bv             [S,Dout]
    att = softmax_k(mask(qs . k_ / sqrt(Din)))   [q,k,w]
    zs  = (att @ vs) @ Wz + bz       [S,W,Dout]

Device computes everything in the k-on-partitions (transposed) layout and
leaves softmax normalization to the host:
    kT    = Wk^T @ dataT  (+bk)       [Din,S]   (dataT host-pretransposed)
    vsT   = Wv^T @ dataT  (+bv)       [Dout,S]
    WqK   = Wq @ kT                   [DQ,S]    -- scores = query@(Wq@kT)
    bqkT  = (kT^T @ bq)*INV + penT    [S,1]     per-k bias incl. -1e5 on
                                                 wm-masked keys
    VZ    = vsT^T-contract Wz         [S,Dout]  -- zs = P@(vs@Wz)
    per 512-col chunk of (q,w) and k-tile kt:
      sT  = WqK^T @ queryT            [128,512]  (k rows, qw cols)
      PT  = exp(sT*INV + bqkT)        (unnormalized attention, transposed)
      zs_un = PT^T-contract VZ        [128,Dout]
Host finishes: row sums over k, att = PT^T/sum, zs = zs_un/sum + bz, and
overwrites content-masked (cm=0) rows with the uniform-attention result,
matching the reference's softmax over all -1e9 exactly.
"""

from contextlib import ExitStack

import numpy as np

import concourse.bass as bass
import concourse.mybir as mybir
import concourse.tile as tile
from concourse import bacc
from concourse import bass_utils

F32 = mybir.dt.float32
F32R = mybir.dt.float32r

N_CORES = 8
B, S, W, DQ, DIN, DOUT = 16, 512, 8, 256, 512, 512
BPC = B // N_CORES          # batches per core
QW = S * W                  # 4096 flattened (q, w) rows
BIG_WM = 1.0e5              # penalty folded into bqkT for wm-masked keys
INV = 1.0 / float(np.sqrt(np.float32(DIN)))
N_CH = 8                    # qw chunks of 512 per batch


def build():
    nc = bacc.Bacc("TRN2", target_bir_lowering=False, debug=False,
                   num_devices=N_CORES)

    def din(name, shape, dt=F32R):
        return nc.dram_tensor(name, shape, dt, kind="ExternalInput").ap()

    qT = din("queryT", [BPC, DQ, QW])
    dT = din("dataT", [BPC, DIN, S])
    wqt = din("WqT", [DIN, DQ])
    wk = din("Wk", [DIN, DIN])
    wv = din("Wv", [DIN, DOUT])
    wz = din("Wz", [DOUT, DOUT])
    bq = din("bq", [DIN])
    bk = din("bk", [DIN], F32)
    bv = din("bv", [DOUT], F32)
    # host-prescaled per-key bias add-on: -BIG_WM*(1-wm)*INV
    pen = din("penTI", [BPC, S], F32)
    attT_o = nc.dram_tensor("attT", [BPC, S, QW], F32,
                            kind="ExternalOutput").ap()
    zs_o = nc.dram_tensor("zs", [BPC, QW, DOUT], F32,
                          kind="ExternalOutput").ap()

    with tile.TileContext(nc) as tc, ExitStack() as ctx:
        sing = ctx.enter_context(tc.tile_pool(name="sing", bufs=1))
        batch_p = ctx.enter_context(tc.tile_pool(name="batch", bufs=2))
        chunk_p = ctx.enter_context(tc.tile_pool(name="chunk", bufs=3))
        sub_p = ctx.enter_context(tc.tile_pool(name="sub", bufs=3))
        ps_set = ctx.enter_context(
            tc.tile_pool(name="ps_set", bufs=2, space="PSUM"))
        ps_scT = ctx.enter_context(
            tc.tile_pool(name="ps_scT", bufs=4, space="PSUM"))
        ps_zs = ctx.enter_context(
            tc.tile_pool(name="ps_zs", bufs=2, space="PSUM"))

        # batch-0 dataT first so the kT matmuls can start ASAP
        dT_tiles = []
        for b in range(BPC):
            dT_tiles.append(batch_p.tile([128, 4, S], F32R, tag="dT", name="dT_s"))
        nc.sync.dma_start(out=dT_tiles[0],
                          in_=dT[0].rearrange("(t p) k -> p t k", p=128))
        wk_s = sing.tile([128, 4, DIN], F32R)
        nc.sync.dma_start(out=wk_s, in_=wk.rearrange("(t p) i -> p t i", p=128))
        wqt_s = sing.tile([128, 4, DQ], F32R)
        nc.sync.dma_start(out=wqt_s, in_=wqt.rearrange("(t p) d -> p t d", p=128))
        bq_s = sing.tile([128, 4], F32R)
        nc.sync.dma_start(out=bq_s, in_=bq.rearrange("(t p) -> p t", p=128))
        bk_s = sing.tile([128, 4], F32)
        nc.sync.dma_start(out=bk_s, in_=bk.rearrange("(t p) -> p t", p=128))
        wv_s = sing.tile([128, 4, DOUT], F32R)
        nc.sync.dma_start(out=wv_s, in_=wv.rearrange("(t p) z -> p t z", p=128))
        bv_s = sing.tile([128, 4], F32)
        nc.sync.dma_start(out=bv_s, in_=bv.rearrange("(t p) -> p t", p=128))
        wz_s = sing.tile([128, 4, DOUT], F32R)
        nc.sync.dma_start(out=wz_s, in_=wz.rearrange("(t p) z -> p t z", p=128))
        if BPC > 1:
            nc.sync.dma_start(out=dT_tiles[1],
                              in_=dT[1].rearrange("(t p) k -> p t k", p=128))

        for b in range(BPC):
            dT_s = dT_tiles[b]
            penI_s = batch_p.tile([128, 4], F32)
            nc.sync.dma_start(out=penI_s,
                              in_=pen[b].rearrange("(t p) -> p t", p=128))

            # kT[i,k] = sum_j Wk[j,i] dataT[j,k]  (+bk per-partition)
            kT_s = batch_p.tile([128, 4, S], F32R)
            for it in range(4):
                ps = ps_set.tile([128, S], F32, tag="ps")
                for jt in range(4):
                    nc.tensor.matmul(ps, wk_s[:, jt, it * 128:(it + 1) * 128],
                                     dT_s[:, jt, :],
                                     start=(jt == 0), stop=(jt == 3))
                nc.vector.tensor_scalar_add(kT_s[:, it, :], ps,
                                            bk_s[:, it:it + 1])
            # WqK[d,k] = sum_i Wq[d,i] kT[i,k]
            wqk_s = batch_p.tile([128, 2, S], F32R)
            for dt_ in range(2):
                ps = ps_set.tile([128, S], F32, tag="ps")
                for it in range(4):
                    nc.tensor.matmul(ps, wqt_s[:, it, dt_ * 128:(dt_ + 1) * 128],
                                     kT_s[:, it, :],
                                     start=(it == 0), stop=(it == 3))
                nc.vector.tensor_copy(wqk_s[:, dt_, :], ps)
            # bqkT[k] = sum_i kT[i,k] bq[i]; bias = bqkT*INV + penTI
            psb = ps_set.tile([128, 4], F32, tag="ps")
            for kt in range(4):
                for it in range(4):
                    # N=1 moving operand is illegal for fp32r -> plain fp32
                    nc.tensor.matmul(psb[:, kt:kt + 1],
                                     kT_s[:, it, kt * 128:(kt + 1) * 128]
                                     .bitcast(F32),
                                     bq_s[:, it:it + 1].bitcast(F32),
                                     start=(it == 0), stop=(it == 3))
            bqkT_s = batch_p.tile([128, 4], F32)
            nc.vector.tensor_scalar_mul(bqkT_s, psb, INV)
            nc.vector.tensor_add(bqkT_s, bqkT_s, penI_s)
            # vsT[z,k] = sum_j Wv[j,z] dataT[j,k]  (+bv per-partition)
            vsT_s = batch_p.tile([128, 4, S], F32R)
            for zt in range(4):
                ps = ps_set.tile([128, S], F32, tag="ps")
                for jt in range(4):
                    nc.tensor.matmul(ps, wv_s[:, jt, zt * 128:(zt + 1) * 128],
                                     dT_s[:, jt, :],
                                     start=(jt == 0), stop=(jt == 3))
                nc.vector.tensor_scalar_add(vsT_s[:, zt, :], ps,
                                            bv_s[:, zt:zt + 1])
            # VZ[k,z2] = sum_z vsT[z,k] Wz[z,z2]
            vz_s = batch_p.tile([128, 4, DOUT], F32R)
            for kt in range(4):
                ps = ps_set.tile([128, S], F32, tag="ps")
                for zt in range(4):
                    nc.tensor.matmul(ps, vsT_s[:, zt, kt * 128:(kt + 1) * 128],
                                     wz_s[:, zt, :],
                                     start=(zt == 0), stop=(zt == 3))
                nc.vector.tensor_copy(vz_s[:, kt, :], ps)

            # ---- main loop: transposed scores -> exp -> zs ----
            for ch in range(N_CH):
                cols = slice(ch * 512, (ch + 1) * 512)
                qT_c = chunk_p.tile([128, 2, 512], F32R, tag="qTc")
                nc.sync.dma_start(
                    out=qT_c,
                    in_=qT[b].rearrange("(t p) c -> p t c", p=128)[:, :, cols])
                ptc = chunk_p.tile([128, 4, 512], F32R, tag="ptc")
                for kt in range(4):
                    ps_t = ps_scT.tile([128, 512], F32, tag="ps_t")
                    for dt_ in range(2):
                        nc.tensor.matmul(
                            ps_t, wqk_s[:, dt_, kt * 128:(kt + 1) * 128],
                            qT_c[:, dt_, :], start=(dt_ == 0), stop=(dt_ == 1))
                    # PT = exp(sT*INV + bqkT') straight from PSUM
                    nc.scalar.activation(
                        out=ptc[:, kt, :], in_=ps_t,
                        func=mybir.ActivationFunctionType.Exp,
                        bias=bqkT_s[:, kt:kt + 1], scale=INV)
                    nc.sync.dma_start(
                        out=attT_o[b, kt * 128:(kt + 1) * 128, cols],
                        in_=ptc[:, kt, :].bitcast(F32))
                for sl in range(4):
                    pz = ps_zs.tile([128, DOUT], F32, tag="pz")
                    for kt in range(4):
                        nc.tensor.matmul(pz,
                                         ptc[:, kt, sl * 128:(sl + 1) * 128],
                                         vz_s[:, kt, :],
                                         start=(kt == 0), stop=(kt == 3))
                    zs_s = sub_p.tile([128, DOUT], F32)
                    nc.scalar.copy(zs_s, pz)
                    nc.sync.dma_start(
                        out=zs_o[b, ch * 512 + sl * 128:
                                 ch * 512 + (sl + 1) * 128, :],
                        in_=zs_s)

    nc.compile()
    return nc


_NC_CACHE = None


def _get_nc():
    global _NC_CACHE
    if _NC_CACHE is None:
        _NC_CACHE = build()
    return _NC_CACHE


def make_in_maps(query, data, content_mask, Wq, bq, Wk, bk, Wv, bv, Wz, bz):
    query = np.ascontiguousarray(query, dtype=np.float32)
    data = np.ascontiguousarray(data, dtype=np.float32)
    wm_full = np.asarray(content_mask).any(axis=2)
    # device wm-fold assumes every batch has >=1 word-valid key (else the
    # reference row would be uniform while the device row would softmax)
    assert wm_full.any(axis=1).all(), "batch with all-masked words_mask"
    pen_full = (-BIG_WM * INV) * (1.0 - wm_full.astype(np.float32))
    wqt = np.ascontiguousarray(np.asarray(Wq, dtype=np.float32).T)
    in_maps = []
    for c in range(N_CORES):
        sl = slice(c * BPC, (c + 1) * BPC)
        q_c = query[sl].reshape(BPC, QW, DQ)
        in_maps.append({
            "queryT": np.ascontiguousarray(q_c.transpose(0, 2, 1)),
            "dataT": np.ascontiguousarray(data[sl].transpose(0, 2, 1)),
            "WqT": wqt,
            "Wk": np.ascontiguousarray(Wk, dtype=np.float32),
            "Wv": np.ascontiguousarray(Wv, dtype=np.float32),
            "Wz": np.ascontiguousarray(Wz, dtype=np.float32),
            "bq": np.ascontiguousarray(bq, dtype=np.float32),
            "bk": np.ascontiguousarray(bk, dtype=np.float32),
            "bv": np.ascontiguousarray(bv, dtype=np.float32),
            "penTI": np.ascontiguousarray(pen_full[sl]),
        })
    return in_maps


def postprocess(results, data, content_mask, Wv, bv, Wz, bz):
    """Per-core raw outputs -> full (zs, att) with host normalization and
    exact handling of content-masked (cm=0) rows."""
    bz = np.asarray(bz, dtype=np.float32)
    attT = np.concatenate([r["attT"] for r in results], axis=0)  # [B,S,QW]
    zs_raw = np.concatenate([r["zs"] for r in results], axis=0)  # [B,QW,Dout]
    sums = attT.sum(axis=1, dtype=np.float32)                    # [B,QW]
    att_qwk = np.ascontiguousarray(attT.transpose(0, 2, 1))      # [B,QW,S]
    att_qwk /= sums[..., None]
    zs_n = zs_raw / sums[..., None] + bz

    # content-masked rows: reference softmaxes an all -1e9 row -> uniform
    cm = np.asarray(content_mask).reshape(B, QW)
    dead = ~cm
    if dead.any():
        att_qwk[dead] = np.float32(1.0 / S)
        data = np.asarray(data, dtype=np.float32)
        vs_mean = data.mean(axis=1) @ np.asarray(Wv, np.float32) + bv  # [B,Dout]
        zs_dead = vs_mean @ np.asarray(Wz, np.float32) + bz            # [B,Dout]
        bidx = np.nonzero(dead)[0]
        zs_n[dead] = zs_dead[bidx]

    att = np.ascontiguousarray(
        att_qwk.reshape(B, S, W, S).transpose(0, 1, 3, 2)).astype(np.float32)
    zs = zs_n.reshape(B, S, W, DOUT).astype(np.float32)
    return zs, att


def kernel(query, data, content_mask, Wq, bq, Wk, bk, Wv, bv, Wz, bz):
    nc = _get_nc()
    in_maps = make_in_maps(query, data, content_mask, Wq, bq, Wk, bk,
                           Wv, bv, Wz, bz)
    res = bass_utils.run_bass_kernel_spmd(nc, in_maps,
                                          core_ids=list(range(N_CORES)),
                                          trace=False)
    return postprocess(res.results, data, content_mask, Wv, bv, Wz, bz)


# revision 12
# speedup vs baseline: 1.4076x; 1.2267x over previous
"""Trainium2 Bass kernel for nn_Attention_34119220199768 (sparse attention).

Data-parallel over batch B=16 across 8 NeuronCores (2 batches/core).

Per batch b the reference computes
    qs  = query @ Wq + bq            [S,W,Din]
    k_  = data @ Wk + bk             [S,Din]
    vs  = data @ Wv + bv             [S,Dout]
    att = softmax_k(mask(qs . k_ / sqrt(Din)))   [q,k,w]
    zs  = (att @ vs) @ Wz + bz       [S,W,Dout]

Device computes everything in the k-on-partitions (transposed) layout and
leaves softmax normalization to the host:
    kT    = Wk^T @ dataT  (+bk)       [Din,S]   (dataT host-pretransposed)
    vsT   = Wv^T @ dataT  (+bv)       [Dout,S]
    WqK   = Wq @ kT                   [DQ,S]    -- scores = query@(Wq@kT)
    bqkT  = (kT^T @ bq)*INV + penT    [S,1]     per-k bias incl. -1e5 on
                                                 wm-masked keys
    VZ    = vsT^T-contract Wz         [S,Dout]  -- zs = P@(vs@Wz)
    per 512-col chunk of (q,w) and k-tile kt:
      sT  = WqK^T @ queryT            [128,512]  (k rows, qw cols)
      PT  = exp(sT*INV + bqkT)        (unnormalized attention, transposed)
      zs_un = PT^T-contract VZ        [128,Dout]
Host finishes: row sums over k, att = PT^T/sum, zs = zs_un/sum + bz, and
overwrites content-masked (cm=0) rows with the uniform-attention result,
matching the reference's softmax over all -1e9 exactly.
"""

from contextlib import ExitStack

import numpy as np

import concourse.bass as bass
import concourse.mybir as mybir
import concourse.tile as tile
from concourse import bacc
from concourse import bass_utils

F32 = mybir.dt.float32
F32R = mybir.dt.float32r

N_CORES = 8
B, S, W, DQ, DIN, DOUT = 16, 512, 8, 256, 512, 512
BPC = B // N_CORES          # batches per core
QW = S * W                  # 4096 flattened (q, w) rows
BIG_WM = 1.0e5              # penalty folded into bqkT for wm-masked keys
INV = 1.0 / float(np.sqrt(np.float32(DIN)))
N_CH = 8                    # qw chunks of 512 per batch


def build():
    nc = bacc.Bacc("TRN2", target_bir_lowering=False, debug=False,
                   num_devices=N_CORES)

    def din(name, shape, dt=F32R):
        return nc.dram_tensor(name, shape, dt, kind="ExternalInput").ap()

    qT = din("queryT", [BPC, DQ, QW])
    dT = din("dataT", [BPC, DIN, S])
    wqt = din("WqT", [DIN, DQ])
    wk = din("Wk", [DIN, DIN])
    wv = din("Wv", [DIN, DOUT])
    wz = din("Wz", [DOUT, DOUT])
    # packed per-partition small vectors: [bqT, bkT, bvT, penTI(b=0..)] as
    # [128, 4] column groups -> one contiguous DMA instead of many 4B-strided
    smalls = din("smalls", [128, 12 + 4 * BPC], F32)
    attT_o = nc.dram_tensor("attT", [BPC, S, QW], F32,
                            kind="ExternalOutput").ap()
    zs_o = nc.dram_tensor("zs", [BPC, QW, DOUT], F32,
                          kind="ExternalOutput").ap()

    with tile.TileContext(nc) as tc, ExitStack() as ctx:
        sing = ctx.enter_context(tc.tile_pool(name="sing", bufs=1))
        batch_p = ctx.enter_context(tc.tile_pool(name="batch", bufs=2))
        chunk_p = ctx.enter_context(tc.tile_pool(name="chunk", bufs=3))
        sub_p = ctx.enter_context(tc.tile_pool(name="sub", bufs=3))
        ps_set = ctx.enter_context(
            tc.tile_pool(name="ps_set", bufs=2, space="PSUM"))
        ps_scT = ctx.enter_context(
            tc.tile_pool(name="ps_scT", bufs=4, space="PSUM"))
        ps_zs = ctx.enter_context(
            tc.tile_pool(name="ps_zs", bufs=2, space="PSUM"))

        # batch-0 dataT first so the kT matmuls can start ASAP
        dT_tiles = []
        for b in range(BPC):
            dT_tiles.append(batch_p.tile([128, 4, S], F32R, tag="dT", name="dT_s"))
        nc.sync.dma_start(out=dT_tiles[0],
                          in_=dT[0].rearrange("(t p) k -> p t k", p=128))
        wk_s = sing.tile([128, 4, DIN], F32R)
        nc.sync.dma_start(out=wk_s, in_=wk.rearrange("(t p) i -> p t i", p=128))
        wqt_s = sing.tile([128, 4, DQ], F32R)
        nc.sync.dma_start(out=wqt_s, in_=wqt.rearrange("(t p) d -> p t d", p=128))
        sm_s = sing.tile([128, 12 + 4 * BPC], F32)
        nc.sync.dma_start(out=sm_s, in_=smalls)
        bq_s = sm_s[:, 0:4]
        bk_s = sm_s[:, 4:8]
        bv_s = sm_s[:, 8:12]
        wv_s = sing.tile([128, 4, DOUT], F32R)
        nc.sync.dma_start(out=wv_s, in_=wv.rearrange("(t p) z -> p t z", p=128))
        wz_s = sing.tile([128, 4, DOUT], F32R)
        nc.sync.dma_start(out=wz_s, in_=wz.rearrange("(t p) z -> p t z", p=128))
        if BPC > 1:
            nc.sync.dma_start(out=dT_tiles[1],
                              in_=dT[1].rearrange("(t p) k -> p t k", p=128))

        for b in range(BPC):
            dT_s = dT_tiles[b]
            penI_s = sm_s[:, 12 + 4 * b:12 + 4 * (b + 1)]

            # kT[i,k] = sum_j Wk[j,i] dataT[j,k]  (+bk per-partition)
            kT_s = batch_p.tile([128, 4, S], F32R, bufs=1)
            for it in range(4):
                ps = ps_set.tile([128, S], F32, tag="ps")
                for jt in range(4):
                    nc.tensor.matmul(ps, wk_s[:, jt, it * 128:(it + 1) * 128],
                                     dT_s[:, jt, :],
                                     start=(jt == 0), stop=(jt == 3))
                nc.vector.tensor_scalar_add(kT_s[:, it, :], ps,
                                            bk_s[:, it:it + 1])
            # WqK[d,k] = sum_i Wq[d,i] kT[i,k]
            wqk_s = batch_p.tile([128, 2, S], F32R)
            for dt_ in range(2):
                ps = ps_set.tile([128, S], F32, tag="ps")
                for it in range(4):
                    nc.tensor.matmul(ps, wqt_s[:, it, dt_ * 128:(dt_ + 1) * 128],
                                     kT_s[:, it, :],
                                     start=(it == 0), stop=(it == 3))
                nc.vector.tensor_copy(wqk_s[:, dt_, :], ps)
            # bqkT[k] = sum_i kT[i,k] bq[i]; bias = bqkT*INV + penTI
            psb = ps_set.tile([128, 4], F32, tag="ps")
            for kt in range(4):
                for it in range(4):
                    # N=1 moving operand is illegal for fp32r -> plain fp32
                    nc.tensor.matmul(psb[:, kt:kt + 1],
                                     kT_s[:, it, kt * 128:(kt + 1) * 128]
                                     .bitcast(F32),
                                     bq_s[:, it:it + 1].bitcast(F32),
                                     start=(it == 0), stop=(it == 3))
            bqkT_s = batch_p.tile([128, 4], F32)
            nc.vector.tensor_scalar_mul(bqkT_s, psb, INV)
            nc.vector.tensor_add(bqkT_s, bqkT_s, penI_s)
            # vsT[z,k] = sum_j Wv[j,z] dataT[j,k]  (+bv per-partition)
            vsT_s = batch_p.tile([128, 4, S], F32R, bufs=1)
            for zt in range(4):
                ps = ps_set.tile([128, S], F32, tag="ps")
                for jt in range(4):
                    nc.tensor.matmul(ps, wv_s[:, jt, zt * 128:(zt + 1) * 128],
                                     dT_s[:, jt, :],
                                     start=(jt == 0), stop=(jt == 3))
                nc.vector.tensor_scalar_add(vsT_s[:, zt, :], ps,
                                            bv_s[:, zt:zt + 1])
            # VZ[k,z2] = sum_z vsT[z,k] Wz[z,z2]
            vz_s = batch_p.tile([128, 4, DOUT], F32R)
            for kt in range(4):
                ps = ps_set.tile([128, S], F32, tag="ps")
                for zt in range(4):
                    nc.tensor.matmul(ps, vsT_s[:, zt, kt * 128:(kt + 1) * 128],
                                     wz_s[:, zt, :],
                                     start=(zt == 0), stop=(zt == 3))
                nc.vector.tensor_copy(vz_s[:, kt, :], ps)

            # ---- main loop: transposed scores -> exp -> zs ----
            for ch in range(N_CH):
                cols = slice(ch * 512, (ch + 1) * 512)
                qT_c = chunk_p.tile([128, 2, 512], F32R, tag="qTc", bufs=5)
                # input DMA on the idle gpsimd queue so it is never
                # head-of-line blocked behind output DMAs on sync
                nc.gpsimd.dma_start(
                    out=qT_c,
                    in_=qT[b].rearrange("(t p) c -> p t c", p=128)[:, :, cols])
                ptc = chunk_p.tile([128, 4, 512], F32R, tag="ptc", bufs=4)
                for kt in range(4):
                    ps_t = ps_scT.tile([128, 512], F32, tag="ps_t")
                    for dt_ in range(2):
                        nc.tensor.matmul(
                            ps_t, wqk_s[:, dt_, kt * 128:(kt + 1) * 128],
                            qT_c[:, dt_, :], start=(dt_ == 0), stop=(dt_ == 1))
                    # PT = exp(sT*INV + bqkT') straight from PSUM
                    nc.scalar.activation(
                        out=ptc[:, kt, :], in_=ps_t,
                        func=mybir.ActivationFunctionType.Exp,
                        bias=bqkT_s[:, kt:kt + 1], scale=INV)
                nc.sync.dma_start(
                    out=attT_o[b].rearrange("(t p) c -> p t c", p=128)
                    [:, :, cols],
                    in_=ptc.bitcast(F32))
                zs_c = chunk_p.tile([128, 4, DOUT], F32, tag="zsc", bufs=3)
                for sl in range(4):
                    pz = ps_zs.tile([128, DOUT], F32, tag="pz")
                    for kt in range(4):
                        nc.tensor.matmul(pz,
                                         ptc[:, kt, sl * 128:(sl + 1) * 128],
                                         vz_s[:, kt, :],
                                         start=(kt == 0), stop=(kt == 3))
                    nc.vector.tensor_copy(zs_c[:, sl, :], pz)
                nc.sync.dma_start(
                    out=zs_o[b][ch * 512:(ch + 1) * 512, :]
                    .rearrange("(s p) z -> p s z", p=128),
                    in_=zs_c)

    nc.compile()
    return nc


_NC_CACHE = None


def _get_nc():
    global _NC_CACHE
    if _NC_CACHE is None:
        _NC_CACHE = build()
    return _NC_CACHE


def make_in_maps(query, data, content_mask, Wq, bq, Wk, bk, Wv, bv, Wz, bz):
    query = np.ascontiguousarray(query, dtype=np.float32)
    data = np.ascontiguousarray(data, dtype=np.float32)
    wm_full = np.asarray(content_mask).any(axis=2)
    # device wm-fold assumes every batch has >=1 word-valid key (else the
    # reference row would be uniform while the device row would softmax)
    assert wm_full.any(axis=1).all(), "batch with all-masked words_mask"
    pen_full = (-BIG_WM * INV) * (1.0 - wm_full.astype(np.float32))
    wqt = np.ascontiguousarray(np.asarray(Wq, dtype=np.float32).T)
    bq32 = np.asarray(bq, np.float32).reshape(4, 128).T
    bk32 = np.asarray(bk, np.float32).reshape(4, 128).T
    bv32 = np.asarray(bv, np.float32).reshape(4, 128).T
    in_maps = []
    for c in range(N_CORES):
        sl = slice(c * BPC, (c + 1) * BPC)
        q_c = query[sl].reshape(BPC, QW, DQ)
        pens = [pen_full[c * BPC + b].reshape(4, 128).T for b in range(BPC)]
        smalls = np.ascontiguousarray(
            np.concatenate([bq32, bk32, bv32] + pens, axis=1))
        in_maps.append({
            "queryT": np.ascontiguousarray(q_c.transpose(0, 2, 1)),
            "dataT": np.ascontiguousarray(data[sl].transpose(0, 2, 1)),
            "WqT": wqt,
            "Wk": np.ascontiguousarray(Wk, dtype=np.float32),
            "Wv": np.ascontiguousarray(Wv, dtype=np.float32),
            "Wz": np.ascontiguousarray(Wz, dtype=np.float32),
            "smalls": smalls,
        })
    return in_maps


def postprocess(results, data, content_mask, Wv, bv, Wz, bz):
    """Per-core raw outputs -> full (zs, att) with host normalization and
    exact handling of content-masked (cm=0) rows."""
    bz = np.asarray(bz, dtype=np.float32)
    attT = np.concatenate([r["attT"] for r in results], axis=0)  # [B,S,QW]
    zs_raw = np.concatenate([r["zs"] for r in results], axis=0)  # [B,QW,Dout]
    sums = attT.sum(axis=1, dtype=np.float32)                    # [B,QW]
    att_qwk = np.ascontiguousarray(attT.transpose(0, 2, 1))      # [B,QW,S]
    att_qwk /= sums[..., None]
    zs_n = zs_raw / sums[..., None] + bz

    # content-masked rows: reference softmaxes an all -1e9 row -> uniform
    cm = np.asarray(content_mask).reshape(B, QW)
    dead = ~cm
    if dead.any():
        att_qwk[dead] = np.float32(1.0 / S)
        data = np.asarray(data, dtype=np.float32)
        vs_mean = data.mean(axis=1) @ np.asarray(Wv, np.float32) + bv  # [B,Dout]
        zs_dead = vs_mean @ np.asarray(Wz, np.float32) + bz            # [B,Dout]
        bidx = np.nonzero(dead)[0]
        zs_n[dead] = zs_dead[bidx]

    att = np.ascontiguousarray(
        att_qwk.reshape(B, S, W, S).transpose(0, 1, 3, 2)).astype(np.float32)
    zs = zs_n.reshape(B, S, W, DOUT).astype(np.float32)
    return zs, att


def kernel(query, data, content_mask, Wq, bq, Wk, bk, Wv, bv, Wz, bz):
    nc = _get_nc()
    in_maps = make_in_maps(query, data, content_mask, Wq, bq, Wk, bk,
                           Wv, bv, Wz, bz)
    res = bass_utils.run_bass_kernel_spmd(nc, in_maps,
                                          core_ids=list(range(N_CORES)),
                                          trace=False)
    return postprocess(res.results, data, content_mask, Wv, bv, Wz, bz)


# revision 13
# speedup vs baseline: 1.4084x; 1.0006x over previous
"""Trainium2 Bass kernel for nn_Attention_34119220199768 (sparse attention).

Data-parallel over batch B=16 across 8 NeuronCores (2 batches/core).

Per batch b the reference computes
    qs  = query @ Wq + bq            [S,W,Din]
    k_  = data @ Wk + bk             [S,Din]
    vs  = data @ Wv + bv             [S,Dout]
    att = softmax_k(mask(qs . k_ / sqrt(Din)))   [q,k,w]
    zs  = (att @ vs) @ Wz + bz       [S,W,Dout]

Device computes everything in the k-on-partitions (transposed) layout and
leaves softmax normalization to the host:
    kT    = Wk^T @ dataT  (+bk)       [Din,S]   (dataT host-pretransposed)
    vsT   = Wv^T @ dataT  (+bv)       [Dout,S]
    WqK   = Wq @ kT                   [DQ,S]    -- scores = query@(Wq@kT)
    bqkT  = (kT^T @ bq)*INV + penT    [S,1]     per-k bias incl. -1e5 on
                                                 wm-masked keys
    VZ    = vsT^T-contract Wz         [S,Dout]  -- zs = P@(vs@Wz)
    per 512-col chunk of (q,w) and k-tile kt:
      sT  = WqK^T @ queryT            [128,512]  (k rows, qw cols)
      PT  = exp(sT*INV + bqkT)        (unnormalized attention, transposed)
      zs_un = PT^T-contract VZ        [128,Dout]
Host finishes: row sums over k, att = PT^T/sum, zs = zs_un/sum + bz, and
overwrites content-masked (cm=0) rows with the uniform-attention result,
matching the reference's softmax over all -1e9 exactly.
"""

from contextlib import ExitStack

import numpy as np

import concourse.bass as bass
import concourse.mybir as mybir
import concourse.tile as tile
from concourse import bacc
from concourse import bass_utils
from concourse.tile_rust import add_dep_helper

F32 = mybir.dt.float32
F32R = mybir.dt.float32r

N_CORES = 8
B, S, W, DQ, DIN, DOUT = 16, 512, 8, 256, 512, 512
BPC = B // N_CORES          # batches per core
QW = S * W                  # 4096 flattened (q, w) rows
BIG_WM = 1.0e5              # penalty folded into bqkT for wm-masked keys
INV = 1.0 / float(np.sqrt(np.float32(DIN)))
N_CH = 8                    # qw chunks of 512 per batch


def build():
    nc = bacc.Bacc("TRN2", target_bir_lowering=False, debug=False,
                   num_devices=N_CORES)

    def din(name, shape, dt=F32R):
        return nc.dram_tensor(name, shape, dt, kind="ExternalInput").ap()

    qT = din("queryT", [BPC, DQ, QW])
    dT = din("dataT", [BPC, DIN, S])
    wqt = din("WqT", [DIN, DQ])
    wk = din("Wk", [DIN, DIN])
    wv = din("Wv", [DIN, DOUT])
    wz = din("Wz", [DOUT, DOUT])
    # packed per-partition small vectors: [bqT, bkT, bvT, penTI(b=0..)] as
    # [128, 4] column groups -> one contiguous DMA instead of many 4B-strided
    smalls = din("smalls", [128, 12 + 4 * BPC], F32)
    attT_o = nc.dram_tensor("attT", [BPC, S, QW], F32,
                            kind="ExternalOutput").ap()
    zs_o = nc.dram_tensor("zs", [BPC, QW, DOUT], F32,
                          kind="ExternalOutput").ap()

    with tile.TileContext(nc) as tc, ExitStack() as ctx:
        sing = ctx.enter_context(tc.tile_pool(name="sing", bufs=1))
        batch_p = ctx.enter_context(tc.tile_pool(name="batch", bufs=2))
        chunk_p = ctx.enter_context(tc.tile_pool(name="chunk", bufs=3))
        sub_p = ctx.enter_context(tc.tile_pool(name="sub", bufs=3))
        ps_set = ctx.enter_context(
            tc.tile_pool(name="ps_set", bufs=2, space="PSUM"))
        ps_scT = ctx.enter_context(
            tc.tile_pool(name="ps_scT", bufs=4, space="PSUM"))
        ps_zs = ctx.enter_context(
            tc.tile_pool(name="ps_zs", bufs=2, space="PSUM"))

        # batch-0 dataT first so the kT matmuls can start ASAP
        dT_tiles = []
        for b in range(BPC):
            dT_tiles.append(batch_p.tile([128, 4, S], F32R, tag="dT", name="dT_s"))
        nc.sync.dma_start(out=dT_tiles[0],
                          in_=dT[0].rearrange("(t p) k -> p t k", p=128))
        wk_s = sing.tile([128, 4, DIN], F32R)
        nc.sync.dma_start(out=wk_s, in_=wk.rearrange("(t p) i -> p t i", p=128))
        wqt_s = sing.tile([128, 4, DQ], F32R)
        nc.sync.dma_start(out=wqt_s, in_=wqt.rearrange("(t p) d -> p t d", p=128))
        sm_s = sing.tile([128, 12 + 4 * BPC], F32)
        nc.sync.dma_start(out=sm_s, in_=smalls)
        bq_s = sm_s[:, 0:4]
        bk_s = sm_s[:, 4:8]
        bv_s = sm_s[:, 8:12]
        wv_s = sing.tile([128, 4, DOUT], F32R)
        dma_wv = nc.sync.dma_start(out=wv_s,
                                   in_=wv.rearrange("(t p) z -> p t z", p=128))
        wz_s = sing.tile([128, 4, DOUT], F32R)
        dma_wz = nc.sync.dma_start(out=wz_s,
                                   in_=wz.rearrange("(t p) z -> p t z", p=128))
        dma_dT1 = None
        if BPC > 1:
            dma_dT1 = nc.sync.dma_start(
                out=dT_tiles[1],
                in_=dT[1].rearrange("(t p) k -> p t k", p=128))
        head_anchors = []

        for b in range(BPC):
            dT_s = dT_tiles[b]
            penI_s = sm_s[:, 12 + 4 * b:12 + 4 * (b + 1)]

            # kT[i,k] = sum_j Wk[j,i] dataT[j,k]  (+bk per-partition)
            kT_s = batch_p.tile([128, 4, S], F32R, bufs=1)
            for it in range(4):
                ps = ps_set.tile([128, S], F32, tag="ps")
                for jt in range(4):
                    nc.tensor.matmul(ps, wk_s[:, jt, it * 128:(it + 1) * 128],
                                     dT_s[:, jt, :],
                                     start=(jt == 0), stop=(jt == 3))
                ep = nc.vector.tensor_scalar_add(kT_s[:, it, :], ps,
                                                 bk_s[:, it:it + 1])
                if b == 0:
                    head_anchors.append(ep)
            if b == 0:
                # hold the not-yet-needed bulk input DMAs behind the first
                # kT matmuls so dataT0/Wk don't share DMA bandwidth with them
                add_dep_helper(dma_wv.ins, head_anchors[0].ins,
                               reason="stagger head DMA")
                add_dep_helper(dma_wz.ins, head_anchors[2].ins,
                               reason="stagger head DMA")
                if dma_dT1 is not None:
                    add_dep_helper(dma_dT1.ins, head_anchors[1].ins,
                                   reason="stagger head DMA")
            # WqK[d,k] = sum_i Wq[d,i] kT[i,k]
            wqk_s = batch_p.tile([128, 2, S], F32R)
            for dt_ in range(2):
                ps = ps_set.tile([128, S], F32, tag="ps")
                for it in range(4):
                    nc.tensor.matmul(ps, wqt_s[:, it, dt_ * 128:(dt_ + 1) * 128],
                                     kT_s[:, it, :],
                                     start=(it == 0), stop=(it == 3))
                nc.vector.tensor_copy(wqk_s[:, dt_, :], ps)
            # bqkT[k] = sum_i kT[i,k] bq[i]; bias = bqkT*INV + penTI
            psb = ps_set.tile([128, 4], F32, tag="ps")
            for kt in range(4):
                for it in range(4):
                    # N=1 moving operand is illegal for fp32r -> plain fp32
                    nc.tensor.matmul(psb[:, kt:kt + 1],
                                     kT_s[:, it, kt * 128:(kt + 1) * 128]
                                     .bitcast(F32),
                                     bq_s[:, it:it + 1].bitcast(F32),
                                     start=(it == 0), stop=(it == 3))
            bqkT_s = batch_p.tile([128, 4], F32)
            nc.vector.tensor_scalar_mul(bqkT_s, psb, INV)
            nc.vector.tensor_add(bqkT_s, bqkT_s, penI_s)
            # vsT[z,k] = sum_j Wv[j,z] dataT[j,k]  (+bv per-partition)
            vsT_s = batch_p.tile([128, 4, S], F32R, bufs=1)
            for zt in range(4):
                ps = ps_set.tile([128, S], F32, tag="ps")
                for jt in range(4):
                    nc.tensor.matmul(ps, wv_s[:, jt, zt * 128:(zt + 1) * 128],
                                     dT_s[:, jt, :],
                                     start=(jt == 0), stop=(jt == 3))
                nc.vector.tensor_scalar_add(vsT_s[:, zt, :], ps,
                                            bv_s[:, zt:zt + 1])
            # VZ[k,z2] = sum_z vsT[z,k] Wz[z,z2]
            vz_s = batch_p.tile([128, 4, DOUT], F32R)
            for kt in range(4):
                ps = ps_set.tile([128, S], F32, tag="ps")
                for zt in range(4):
                    nc.tensor.matmul(ps, vsT_s[:, zt, kt * 128:(kt + 1) * 128],
                                     wz_s[:, zt, :],
                                     start=(zt == 0), stop=(zt == 3))
                nc.vector.tensor_copy(vz_s[:, kt, :], ps)

            # ---- main loop: transposed scores -> exp -> zs ----
            for ch in range(N_CH):
                cols = slice(ch * 512, (ch + 1) * 512)
                qT_c = chunk_p.tile([128, 2, 512], F32R, tag="qTc", bufs=5)
                # input DMA on the idle gpsimd queue so it is never
                # head-of-line blocked behind output DMAs on sync
                dma_q = nc.gpsimd.dma_start(
                    out=qT_c,
                    in_=qT[b].rearrange("(t p) c -> p t c", p=128)[:, :, cols])
                if b == 0 and 1 <= ch <= 4:
                    add_dep_helper(dma_q.ins, head_anchors[ch - 1].ins,
                                   reason="stagger head DMA")
                ptc = chunk_p.tile([128, 4, 512], F32R, tag="ptc", bufs=4)
                for kt in range(4):
                    ps_t = ps_scT.tile([128, 512], F32, tag="ps_t")
                    for dt_ in range(2):
                        nc.tensor.matmul(
                            ps_t, wqk_s[:, dt_, kt * 128:(kt + 1) * 128],
                            qT_c[:, dt_, :], start=(dt_ == 0), stop=(dt_ == 1))
                    # PT = exp(sT*INV + bqkT') straight from PSUM
                    nc.scalar.activation(
                        out=ptc[:, kt, :], in_=ps_t,
                        func=mybir.ActivationFunctionType.Exp,
                        bias=bqkT_s[:, kt:kt + 1], scale=INV)
                if b == BPC - 1 and ch == N_CH - 1:
                    for kt in range(4):
                        nc.sync.dma_start(
                            out=attT_o[b, kt * 128:(kt + 1) * 128, cols],
                            in_=ptc[:, kt, :].bitcast(F32))
                else:
                    nc.sync.dma_start(
                        out=attT_o[b].rearrange("(t p) c -> p t c", p=128)
                        [:, :, cols],
                        in_=ptc.bitcast(F32))
                zs_c = chunk_p.tile([128, 4, DOUT], F32, tag="zsc", bufs=3)
                for sl in range(4):
                    pz = ps_zs.tile([128, DOUT], F32, tag="pz")
                    for kt in range(4):
                        nc.tensor.matmul(pz,
                                         ptc[:, kt, sl * 128:(sl + 1) * 128],
                                         vz_s[:, kt, :],
                                         start=(kt == 0), stop=(kt == 3))
                    nc.vector.tensor_copy(zs_c[:, sl, :], pz)
                if b == BPC - 1 and ch == N_CH - 1:
                    for sl in range(4):
                        nc.sync.dma_start(
                            out=zs_o[b, ch * 512 + sl * 128:
                                     ch * 512 + (sl + 1) * 128, :],
                            in_=zs_c[:, sl, :])
                else:
                    nc.sync.dma_start(
                        out=zs_o[b][ch * 512:(ch + 1) * 512, :]
                        .rearrange("(s p) z -> p s z", p=128),
                        in_=zs_c)

    nc.compile()
    return nc


_NC_CACHE = None


def _get_nc():
    global _NC_CACHE
    if _NC_CACHE is None:
        _NC_CACHE = build()
    return _NC_CACHE


def make_in_maps(query, data, content_mask, Wq, bq, Wk, bk, Wv, bv, Wz, bz):
    query = np.ascontiguousarray(query, dtype=np.float32)
    data = np.ascontiguousarray(data, dtype=np.float32)
    wm_full = np.asarray(content_mask).any(axis=2)
    # device wm-fold assumes every batch has >=1 word-valid key (else the
    # reference row would be uniform while the device row would softmax)
    assert wm_full.any(axis=1).all(), "batch with all-masked words_mask"
    pen_full = (-BIG_WM * INV) * (1.0 - wm_full.astype(np.float32))
    wqt = np.ascontiguousarray(np.asarray(Wq, dtype=np.float32).T)
    bq32 = np.asarray(bq, np.float32).reshape(4, 128).T
    bk32 = np.asarray(bk, np.float32).reshape(4, 128).T
    bv32 = np.asarray(bv, np.float32).reshape(4, 128).T
    in_maps = []
    for c in range(N_CORES):
        sl = slice(c * BPC, (c + 1) * BPC)
        q_c = query[sl].reshape(BPC, QW, DQ)
        pens = [pen_full[c * BPC + b].reshape(4, 128).T for b in range(BPC)]
        smalls = np.ascontiguousarray(
            np.concatenate([bq32, bk32, bv32] + pens, axis=1))
        in_maps.append({
            "queryT": np.ascontiguousarray(q_c.transpose(0, 2, 1)),
            "dataT": np.ascontiguousarray(data[sl].transpose(0, 2, 1)),
            "WqT": wqt,
            "Wk": np.ascontiguousarray(Wk, dtype=np.float32),
            "Wv": np.ascontiguousarray(Wv, dtype=np.float32),
            "Wz": np.ascontiguousarray(Wz, dtype=np.float32),
            "smalls": smalls,
        })
    return in_maps


def postprocess(results, data, content_mask, Wv, bv, Wz, bz):
    """Per-core raw outputs -> full (zs, att) with host normalization and
    exact handling of content-masked (cm=0) rows."""
    bz = np.asarray(bz, dtype=np.float32)
    attT = np.concatenate([r["attT"] for r in results], axis=0)  # [B,S,QW]
    zs_raw = np.concatenate([r["zs"] for r in results], axis=0)  # [B,QW,Dout]
    sums = attT.sum(axis=1, dtype=np.float32)                    # [B,QW]
    att_qwk = np.ascontiguousarray(attT.transpose(0, 2, 1))      # [B,QW,S]
    att_qwk /= sums[..., None]
    zs_n = zs_raw / sums[..., None] + bz

    # content-masked rows: reference softmaxes an all -1e9 row -> uniform
    cm = np.asarray(content_mask).reshape(B, QW)
    dead = ~cm
    if dead.any():
        att_qwk[dead] = np.float32(1.0 / S)
        data = np.asarray(data, dtype=np.float32)
        vs_mean = data.mean(axis=1) @ np.asarray(Wv, np.float32) + bv  # [B,Dout]
        zs_dead = vs_mean @ np.asarray(Wz, np.float32) + bz            # [B,Dout]
        bidx = np.nonzero(dead)[0]
        zs_n[dead] = zs_dead[bidx]

    att = np.ascontiguousarray(
        att_qwk.reshape(B, S, W, S).transpose(0, 1, 3, 2)).astype(np.float32)
    zs = zs_n.reshape(B, S, W, DOUT).astype(np.float32)
    return zs, att


def kernel(query, data, content_mask, Wq, bq, Wk, bk, Wv, bv, Wz, bz):
    nc = _get_nc()
    in_maps = make_in_maps(query, data, content_mask, Wq, bq, Wk, bk,
                           Wv, bv, Wz, bz)
    res = bass_utils.run_bass_kernel_spmd(nc, in_maps,
                                          core_ids=list(range(N_CORES)),
                                          trace=False)
    return postprocess(res.results, data, content_mask, Wv, bv, Wz, bz)
